# revision 3
# baseline (speedup 1.0000x reference)
"""Causal multi-head attention with RoPE for TRN2 (Bass/Tile), 8 NeuronCores.

Problem: y = (softmax(causal(rope(x@Wq) @ rope(x@Wk)^T / sqrt(dh))) @ (x@Wv)) @ Wo
  B=4, T=2048, D=2048, H=16 heads, dh=128, fp32 I/O.

Sharding: 4-way batch data-parallel x 2-way head tensor-parallel.
  Core c: batch b = c//2, head group g = c%2 (heads 8g..8g+7).
  Each core computes a partial y[b] (its 8 heads' contribution through Wo);
  the host sums the two partials per batch in fp32.

All matmuls run in fp16 (full PE rate; fp32 PSUM accumulation), everything
kept transposed so no on-chip transposes are needed (see stage comments).

Key layout/scheduling choices (measured ~10% faster than the previous rev):
  - Q^T/K^T for ALL heads stay resident in SBUF (no DRAM roundtrip); stage A
    is ordered V(g0) -> QK(h0..3) -> V(g1) -> QK(h4..7) so everything fits in
    SBUF and stage B's inputs are ready earliest. One shared PSUM pool for
    stage A (pv 3 banks + paired q-tiles 4 banks) avoids pool-transition
    drains.
  - QK projections accumulate jt-PAIRS into one [128,1024] 2-bank PSUM tile;
    the RoPE drain then runs 1024 wide, halving DVE instruction count.
  - Causal trimming: diagonal k-chunks only matmul/exp/accumulate the valid
    q-range (saves ~15% of S/PV/exp work). The masked-out prefix of each
    diagonal es tile is zeroed ONCE per rep (tiles are tagged per diagonal
    offset), so full-width rowsum reads see exact zeros; the 128x128
    triangle mask is applied in-place on just the diagonal block.
  - Stage B full k-chunks are processed in PAIRS: two S matmuls into one
    [128,1024] PSUM tile and a single wide exp (halves ACT instruction
    count where exp throughput is the stage bottleneck).
  - Softmax denominators come from a DVE pair-tree + one ones-matmul per
    quad accumulated alongside P@V; no max-subtraction (logits are O(1)).
  - cos/sin tables in fp16; y output in fp16 (halves output DMA).
  - Stage C drains PSUM via the scalar engine, keeping DVE free.
RoPE pair partners are pre-permuted into partition halves (even dh dims ->
partitions 0..63, odd -> 64..127) via a host-side column permutation of
Wq/Wk, making rotate-half a uniform +-64-partition shift on chip.
"""

import numpy as np

import concourse.bass as bass
import concourse.tile as tile
from concourse import bacc, mybir
from concourse.bass import ts
from concourse.bass_utils import run_bass_kernel_spmd

B, T, D = 4, 2048, 2048
H = 16
DH = 128
THETA = 10000.0
NCORES = 8
HPC = H // 2  # heads per core (2-way head TP)
P = 128
TQ = 512  # q-tile width
F16 = mybir.dt.float16
F32 = mybir.dt.float32


def build(t=T, d=D, hpc=HPC, reps=1, stages="ABC", mmdt=None):
    """Build the per-core Bass program (same program on all cores)."""
    nc = bacc.Bacc("TRN2", target_bir_lowering=False, debug=False)
    MMDT = mmdt or F16
    dc = d // P  # contraction chunks for projections
    tc_n = t // P  # token chunks (k-chunks in attention)
    ntq = t // TQ  # q tiles
    vg = max(1, hpc // 4)  # V head-groups of up to 4 heads (N=512)
    vgh = hpc // vg  # heads per V group
    vgw = vgh * DH  # V group width
    hpg = hpc // vg  # heads per V group (= vgh)

    xt = nc.dram_tensor("xt", [d, t], MMDT, kind="ExternalInput").ap()
    wq = nc.dram_tensor("wq", [hpc, d, DH], MMDT, kind="ExternalInput").ap()
    wk = nc.dram_tensor("wk", [hpc, d, DH], MMDT, kind="ExternalInput").ap()
    wv = nc.dram_tensor("wv", [vg, d, vgw], MMDT, kind="ExternalInput").ap()
    wo = nc.dram_tensor("wo", [hpc * DH, d], MMDT, kind="ExternalInput").ap()
    cos = nc.dram_tensor("cos", [P, t], F16, kind="ExternalInput").ap()
    sin = nc.dram_tensor("sin", [P, t], F16, kind="ExternalInput").ap()
    maskt = nc.dram_tensor("maskt", [P, P], MMDT, kind="ExternalInput").ap()
    ones = nc.dram_tensor("ones", [P, P], MMDT, kind="ExternalInput").ap()
    y = nc.dram_tensor("y", [d, t], F16, kind="ExternalOutput").ap()

    with tile.TileContext(nc) as tc:
        import contextlib

        with tc.tile_pool(name="const", bufs=1) as constp:
            cos_sb = constp.tile([P, t], F16, tag="cos")
            sin_sb = constp.tile([P, t], F16, tag="sin")
            maskt_sb = constp.tile([P, P], MMDT, tag="maskt")
            ones_sb = constp.tile([P, P], MMDT, tag="ones")
            nc.sync.dma_start(cos_sb[:], cos)
            nc.sync.dma_start(sin_sb[:], sin)
            nc.sync.dma_start(maskt_sb[:], maskt)
            nc.sync.dma_start(ones_sb[:], ones)

            loop_cm = tc.For_i(0, reps, 1) if reps > 1 else contextlib.nullcontext()
            with loop_cm, tc.tile_pool(name="warm", bufs=1) as warmp:
                # q at index h, k at index hpc+h
                qkt_sb = warmp.tile([P, 2 * hpc, t], MMDT, tag="qkt")
                v_sbs = [
                    warmp.tile([P, tc_n, vgw], MMDT, tag=f"v{g}", name=f"v{g}")
                    for g in range(vg)
                ]

                # ------------- Stage A: QKV projections + RoPE -------------
                with (
                    tc.tile_pool(name="xt", bufs=1) as xtp,
                    tc.tile_pool(name="wv", bufs=1) as wvp,
                    tc.tile_pool(name="wqk", bufs=2) as wqkp,
                    tc.tile_pool(name="adrain", bufs=1) as adp,
                ):
                    xt_sb = xtp.tile([P, dc, t], MMDT, tag="xt")
                    for q4 in range(t // TQ):
                        for c in range(dc):
                            nc.sync.dma_start(
                                xt_sb[:, c, ts(q4, TQ)], xt[ts(c, P), ts(q4, TQ)]
                            )

                    def proj_v(g, apsp):
                        wv_g = wvp.tile([P, dc, vgw], MMDT, tag="wv", bufs=1)
                        nc.sync.dma_start(
                            wv_g[:], wv[g].rearrange("(c p) m -> p c m", p=P)
                        )
                        for tt in range(tc_n):
                            pv = apsp.tile([P, vgw], F32, tag="pv", bufs=3, name="pv")
                            for c in range(dc):
                                nc.tensor.matmul(
                                    pv[:],
                                    xt_sb[:, c, ts(tt, P)],
                                    wv_g[:, c, :],
                                    start=(c == 0),
                                    stop=(c == dc - 1),
                                )
                            nc.scalar.copy(v_sbs[g][:, tt, :], pv[:])

                    def rope_drain2(pq, jp, out_col):
                        # paired drain over [128, 2*TQ]: out = pq*cos + rot(pq)*sin
                        w2 = 2 * TQ
                        rot = adp.tile([P, w2], MMDT, tag="rot")
                        nc.vector.scalar_tensor_tensor(
                            rot[0:64, :],
                            pq[64:128, :],
                            -1.0,
                            sin_sb[0:64, ts(jp, w2)],
                            mybir.AluOpType.mult,
                            mybir.AluOpType.mult,
                        )
                        nc.vector.tensor_mul(
                            rot[64:128, :], pq[0:64, :], sin_sb[64:128, ts(jp, w2)]
                        )
                        t1 = adp.tile([P, w2], MMDT, tag="t1")
                        nc.vector.tensor_mul(t1[:], pq[:], cos_sb[:, ts(jp, w2)])
                        nc.vector.tensor_add(
                            qkt_sb[:, out_col, ts(jp, w2)], t1[:], rot[:]
                        )

                    def proj_qk(h, apsp):
                        for qk, w_ap in ((0, wq[h]), (1, wk[h])):
                            w_sb = wqkp.tile([P, dc, DH], MMDT, tag=f"w{qk}")
                            nc.sync.dma_start(
                                w_sb[:], w_ap.rearrange("(c p) m -> p c m", p=P)
                            )
                            for jp in range(ntq // 2):
                                pq = apsp.tile(
                                    [P, 2 * TQ], F32, tag="pq", bufs=2, name="pq"
                                )
                                for c in range(dc):
                                    for j in range(2):
                                        nc.tensor.matmul(
                                            pq[:, ts(j, TQ)],
                                            w_sb[:, c, :],
                                            xt_sb[:, c, ts(2 * jp + j, TQ)],
                                            start=(c == 0),
                                            stop=(c == dc - 1),
                                        )
                                rope_drain2(pq, jp, qk * hpc + h)

                    with tc.tile_pool(name="aps", bufs=1, space="PSUM") as apsp:
                        for g in range(vg):
                            proj_v(g, apsp)
                            for h in range(g * hpg, (g + 1) * hpg):
                                proj_qk(h, apsp)

                # ------------- Stage B: attention per head -------------
                wop_cm = tc.tile_pool(name="wop", bufs=1)
                wop = wop_cm.__enter__()
                aot_sb = wop.tile([P, hpc, t], MMDT, tag="aot")
                wo_sb = wop.tile([P, hpc, d], MMDT, tag="wo")
                wo_r = wo.rearrange("(h p) n -> p h n", p=P)
                vg_b = vg if "B" in stages else 0

                with (
                    tc.tile_pool(name="exps", bufs=8) as esp,
                    tc.tile_pool(name="etree", bufs=4) as etp,
                    tc.tile_pool(name="small", bufs=4) as smp,
                    tc.tile_pool(name="bpsum", bufs=2, space="PSUM") as bpsp,
                    tc.tile_pool(name="spsum", bufs=4, space="PSUM") as spsp,
                ):
                    # es tiles tagged per diagonal offset o; [0 : o*128) is
                    # zeroed once here and never rewritten, so full-width
                    # reads (pairtree) see exact zeros in the masked region.
                    es_tiles = {}
                    for o in range(1, TQ // P):
                        for b_ in range(2):
                            til = esp.tile(
                                [P, TQ], MMDT, tag=f"es_o{o}", bufs=2, name=f"es_o{o}"
                            )
                            if vg_b:
                                nc.vector.memset(til[:, 0 : o * P], 0.0)
                            es_tiles.setdefault(o, []).append(til)

                    def attn_head_jt(h, jt, qt_col, kt_col, v_sb, hh):
                        op = bpsp.tile([P, TQ], F32, tag="op", name="op")
                        rp = bpsp.tile([P, TQ], F32, tag="rp", name="rp")
                        nch = (jt + 1) * (TQ // P)
                        nquad = nch // 4
                        nfull = jt * (TQ // P)
                        qsl = lambda a: slice(jt * TQ + a, (jt + 1) * TQ)
                        equad = []
                        nq = [0]

                        def quad_mm(e4):
                            nc.tensor.matmul(
                                rp[:],
                                ones_sb[:],
                                e4[:],
                                start=(nq[0] == 0),
                                stop=(nq[0] == nquad - 1),
                            )
                            nq[0] += 1

                        # full chunks in pairs: 2 S-MMs into one 2-bank PSUM
                        # tile, a single wide exp, then 2 PV accumulations
                        for cp in range(0, nfull, 2):
                            sp = spsp.tile([P, 2 * TQ], F32, tag="sp", bufs=2, name="sp")
                            for j in range(2):
                                nc.tensor.matmul(
                                    sp[:, ts(j, TQ)],
                                    qkt_sb[:, kt_col, ts(cp + j, P)],
                                    qkt_sb[:, qt_col, qsl(0)],
                                    start=True,
                                    stop=True,
                                )
                            es = esp.tile([P, 2 * TQ], MMDT, tag="es_pair", bufs=4)
                            nc.scalar.activation(
                                es[:], sp[:], mybir.ActivationFunctionType.Exp
                            )
                            for j in range(2):
                                nc.tensor.matmul(
                                    op[:],
                                    v_sb[:, cp + j, ts(hh, DH)],
                                    es[:, ts(j, TQ)],
                                    start=(cp + j == 0),
                                    stop=False,
                                )
                            e2 = etp.tile([P, TQ], MMDT, tag="e2")
                            nc.vector.tensor_add(
                                e2[:], es[:, 0:TQ], es[:, TQ : 2 * TQ]
                            )
                            equad.append(e2)
                            if len(equad) == 2:
                                e4 = etp.tile([P, TQ], MMDT, tag="e4")
                                nc.vector.tensor_add(e4[:], equad[0][:], equad[1][:])
                                equad = []
                                quad_mm(e4)

                        # diagonal chunks: trimmed singles with triangle mask
                        epair = None
                        for o in range(TQ // P):
                            c = nfull + o
                            a = o * P
                            sp = spsp.tile([P, 2 * TQ], F32, tag="sp", bufs=2, name="sp")
                            nc.tensor.matmul(
                                sp[:, a:TQ],
                                qkt_sb[:, kt_col, ts(c, P)],
                                qkt_sb[:, qt_col, qsl(a)],
                                start=True,
                                stop=True,
                            )
                            if o > 0:
                                es = es_tiles[o][0]
                                es_tiles[o] = es_tiles[o][1:] + [es]
                            else:
                                es = esp.tile([P, TQ], MMDT, tag="es_d0", bufs=3)
                            nc.scalar.activation(
                                es[:, a:TQ],
                                sp[:, a:TQ],
                                mybir.ActivationFunctionType.Exp,
                            )
                            # in-place triangle mask on the 128-col diag block
                            nc.vector.tensor_mul(
                                es[:, a : a + P], es[:, a : a + P], maskt_sb[:]
                            )
                            nc.tensor.matmul(
                                op[:, a:TQ],
                                v_sb[:, c, ts(hh, DH)],
                                es[:, a:TQ],
                                start=(c == 0),
                                stop=(c == nch - 1),
                            )
                            if epair is None:
                                epair = es
                            else:
                                e2 = etp.tile([P, TQ], MMDT, tag="e2")
                                nc.vector.tensor_add(e2[:], epair[:], es[:])
                                epair = None
                                equad.append(e2)
                            if len(equad) == 2:
                                e4 = etp.tile([P, TQ], MMDT, tag="e4")
                                nc.vector.tensor_add(e4[:], equad[0][:], equad[1][:])
                                equad = []
                                quad_mm(e4)
                        rs = smp.tile([P, TQ], F32, tag="rs")
                        nc.vector.reciprocal(rs[:], rp[:])
                        nc.vector.tensor_mul(aot_sb[:, h, ts(jt, TQ)], op[:], rs[:])

                    for g in range(vg_b):
                        v_sb = v_sbs[g]
                        for hh in range(vgh):
                            h = g * vgh + hh
                            for jt in range(ntq):
                                attn_head_jt(h, jt, h, hpc + h, v_sb, hh)
                            nc.sync.dma_start(wo_sb[:, h, :], wo_r[:, h, :])

                # ---------------- Stage C: output projection ----------------
                with (
                    tc.tile_pool(name="cdrain", bufs=3) as cdp,
                    tc.tile_pool(name="cpsum", bufs=2, space="PSUM") as cpsp,
                ):
                    for nt in range(d // P if "C" in stages else 0):
                        yps = []
                        for tq in range(t // TQ):
                            ypt = cpsp.tile([P, TQ], F32, tag=f"yp{tq}", name=f"yp{tq}")
                            yps.append(ypt)
                        for h in range(hpc):
                            for tq in range(t // TQ):
                                nc.tensor.matmul(
                                    yps[tq][:],
                                    wo_sb[:, h, ts(nt, P)],
                                    aot_sb[:, h, ts(tq, TQ)],
                                    start=(h == 0),
                                    stop=(h == hpc - 1),
                                )
                        for tq in range(t // TQ):
                            ytile = cdp.tile([P, TQ], F16, tag="ytile")
                            nc.scalar.copy(ytile[:], yps[tq][:])
                            nc.sync.dma_start(y[ts(nt, P), ts(tq, TQ)], ytile[:])
                wop_cm.__exit__(None, None, None)

    nc.compile()
    return nc


def _rope_tables(t=T):
    """cos/sin in transposed+permuted layout [128, t] (fp16).

    Partition p < 64 holds dh dim 2p (even), p >= 64 holds dh dim 2(p-64)+1;
    pair (2i, 2i+1) shares inv_freq[i], so row p uses inv_freq[p % 64].
    """
    inv_freq = 1.0 / (THETA ** (np.arange(0, DH, 2, dtype=np.float64) / DH))  # [64]
    pos = np.arange(t, dtype=np.float64)
    freqs = pos[None, :] * inv_freq[np.arange(P) % 64][:, None]  # [128, t]
    return (
        np.cos(freqs).astype(np.float16),
        np.sin(freqs).astype(np.float16),
    )


def _perm():
    """Within-head dh permutation: even dims first, then odd dims."""
    return np.concatenate([np.arange(0, DH, 2), np.arange(1, DH, 2)])


def _maskt():
    """maskt[dk, dq] = 1 if dk <= dq else 0 (128x128 triangle)."""
    dk = np.arange(P)[:, None]
    dq = np.arange(P)[None, :]
    return (dk <= dq).astype(np.float16)


def prep_core_inputs(x_b, Wq_g, Wk_g, Wv_g, Wo_g, t=T, hpc=HPC, npdt=np.float16):
    """Host-side input prep for one core.

    x_b: [t, D] (this core's batch); W*_g: this core's head-group slices
    (Wq/Wk/Wv: [D, hpc*DH] columns, Wo: [hpc*DH, D] rows).
    """
    d = x_b.shape[1]
    perm = _perm()
    scale = 1.0 / np.sqrt(DH)
    vg = max(1, hpc // 4)
    vgw = (hpc // vg) * DH

    wq = np.empty((hpc, d, DH), npdt)
    wk = np.empty((hpc, d, DH), npdt)
    for h in range(hpc):
        blk_q = Wq_g[:, h * DH : (h + 1) * DH]
        blk_k = Wk_g[:, h * DH : (h + 1) * DH]
        wq[h] = (blk_q[:, perm] * scale).astype(npdt)
        wk[h] = blk_k[:, perm].astype(npdt)

    cos, sin = _rope_tables(t)
    return {
        "xt": np.ascontiguousarray(x_b.T).astype(npdt),
        "wq": wq,
        "wk": wk,
        "wv": np.ascontiguousarray(
            Wv_g.astype(npdt).reshape(d, vg, vgw).transpose(1, 0, 2)
        ),
        "wo": Wo_g.astype(npdt),
        "cos": cos,
        "sin": sin,
        "maskt": _maskt(),
        "ones": np.ones((P, P), npdt),
    }


def make_in_maps(inputs, npdt=np.float16):
    x, Wq, Wk, Wv, Wo = (
        np.asarray(inputs["x"]),
        np.asarray(inputs["Wq"]),
        np.asarray(inputs["Wk"]),
        np.asarray(inputs["Wv"]),
        np.asarray(inputs["Wo"]),
    )
    in_maps = []
    for c in range(NCORES):
        b, g = c // 2, c % 2
        cols = slice(g * HPC * DH, (g + 1) * HPC * DH)
        in_maps.append(
            prep_core_inputs(
                x[b], Wq[:, cols], Wk[:, cols], Wv[:, cols], Wo[cols, :], npdt=npdt
            )
        )
    return in_maps


def _build_sharded(nc, n_cores=NCORES):
    """Build a reusable jitted 8-core executable (bass2jax multi-core path,
    without output donation so it can be re-invoked for timing)."""
    import jax
    from jax.experimental.shard_map import shard_map
    from jax.sharding import Mesh, NamedSharding, PartitionSpec

    from concourse import bass2jax

    bass2jax.install_neuronx_cc_hook()
    partition_name = nc.partition_id_tensor.name if nc.partition_id_tensor else None
    in_names, out_names, out_avals, zero_outs = [], [], [], []
    for alloc in nc.m.functions[0].allocations:
        if not isinstance(alloc, mybir.MemoryLocationSet):
            continue
        name = alloc.memorylocations[0].name
        if alloc.kind == "ExternalInput":
            if name != partition_name:
                in_names.append(name)
        elif alloc.kind == "ExternalOutput":
            out_names.append(name)
            shape = tuple(alloc.tensor_shape)
            dtype = mybir.dt.np(alloc.dtype)
            out_avals.append(jax.core.ShapedArray(shape, dtype))
            zero_outs.append(np.zeros(shape, dtype))
    n_params = len(in_names)
    all_names = in_names + out_names
    if partition_name is not None:
        all_names = all_names + [partition_name]

    def _body(*args):
        operands = list(args)
        if partition_name is not None:
            operands.append(bass2jax.partition_id_tensor())
        outs = bass2jax._bass_exec_p.bind(
            *operands,
            out_avals=tuple(out_avals),
            in_names=tuple(all_names),
            out_names=tuple(out_names),
            lowering_input_output_aliases=(),
            sim_require_finite=True,
            sim_require_nnan=True,
            nc=nc,
        )
        return tuple(outs)

    def _chain(n):
        def f(*args):
            outs = _body(*args)
            for _ in range(n - 1):
                # 0-valued data dependency on the previous execution's first
                # output forces sequential NEFF executions on-device
                dep = (outs[0].ravel()[0] * 0).astype(args[0].dtype)
                outs = _body(args[0] + dep, *args[1:])
            return outs

        return f

    devices = jax.devices()[:n_cores]
    mesh = Mesh(np.asarray(devices), ("core",))
    in_specs = (PartitionSpec("core"),) * (n_params + len(out_names))
    out_specs = (PartitionSpec("core"),) * len(out_names)

    def _jit(body):
        return jax.jit(
            shard_map(
                body, mesh=mesh, in_specs=in_specs, out_specs=out_specs, check_rep=False
            ),
            keep_unused=True,
        )

    fn = _jit(_body)
    sharding = NamedSharding(mesh, PartitionSpec("core"))
    return fn, _jit, _chain, sharding, in_names, out_names, out_avals, zero_outs


def run_timed(nc, in_maps, reps=6, chain=0, n_cores=NCORES):
    """Run on all cores; return (per-core results, per-exec device ns).

    Per-exec time is estimated as the slope between a chain-of-N jit call and
    a single-exec jit call (axon round-trip and input shipping cancel out).
    """
    import time

    import jax

    fn, _jit, _chain, sharding, in_names, out_names, out_avals, zero_outs = (
        _build_sharded(nc, n_cores)
    )
    concat_in = [
        np.concatenate([np.asarray(in_maps[c][n]) for c in range(n_cores)], axis=0)
        for n in in_names
    ]
    concat_zeros = [
        np.zeros((n_cores * z.shape[0], *z.shape[1:]), z.dtype) for z in zero_outs
    ]
    dev_in = [jax.device_put(a, sharding) for a in concat_in]
    dev_zeros = [jax.device_put(a, sharding) for a in concat_zeros]
    out = jax.block_until_ready(fn(*dev_in, *dev_zeros))

    def _time(f):
        ts = []
        for _ in range(reps):
            t0 = time.perf_counter()
            jax.block_until_ready(f(*dev_in, *dev_zeros))
            ts.append(time.perf_counter() - t0)
        print("rep times (ms):", [f"{x * 1e3:.2f}" for x in ts])
        ts.sort()
        return ts[len(ts) // 2]

    exec_ns = None
    if chain and chain > 1:
        fnc = _jit(_chain(chain))
        jax.block_until_ready(fnc(*dev_in, *dev_zeros))  # compile
        t1 = _time(fn)
        tn = _time(fnc)
        exec_ns = int((tn - t1) / (chain - 1) * 1e9)
        print(f"single call: {t1 * 1e3:.2f} ms, chain-{chain}: {tn * 1e3:.2f} ms")
    else:
        exec_ns = int(_time(fn) * 1e9)
    results = [
        {
            name: np.asarray(out[i]).reshape(n_cores, *out_avals[i].shape)[c]
            for i, name in enumerate(out_names)
        }
        for c in range(n_cores)
    ]
    return results, exec_ns


def kernel(x, Wq, Wk, Wv, Wo):
    nc = build()
    in_maps = make_in_maps({"x": x, "Wq": Wq, "Wk": Wk, "Wv": Wv, "Wo": Wo})
    results = run_bass_kernel_spmd(nc, in_maps, core_ids=list(range(NCORES))).results
    out = np.empty((B, T, D), np.float32)
    for b in range(B):
        out[b] = (
            results[2 * b]["y"].astype(np.float32)
            + results[2 * b + 1]["y"].astype(np.float32)
        ).T
    return out
